# revision 25
# baseline (speedup 1.0000x reference)
"""W-trick fp16 variant with batched DMA and multi-engine scheduling.

Reassociates BOTH bmms through [C,C] intermediates:
  G = x @ qT                    (Gram, 65.5k PE cols)
  E = kw @ G + kb (x) S         (S folded in as a broadcast-stationary matmul
                                 over the un-reduced Pool-engine accumulator)
  A = softmax(E) * gamma/rowsum
  WT = vw^T @ A  (+I on diag)   (8.2k cols; identity folds the residual)
  out = (W+I)^T.T @ x + b2      (65.5k cols; b2 = A^T vb via 1-col matmuls,
                                 added on the Act engine during eviction)

vs the baseline this removes the v-conv (65.5k cols) and its DVE evictions
entirely.  All streams are fp16: same 1 cyc/row PE rate as fp32r but half
the HBM traffic; fp16's 10-bit mantissa matches fp32r's effective multiply
precision so the softmax stays stable (CPU sim: 3.3e-3 rel absmax).
Output is written fp16 and upcast on host.

Scheduling notes (from TimelineSim span analysis):
- HWDGE descriptor generation costs a fixed 625ns per DMA instruction on a
  single shared device -> ~31 batched DMAs instead of 140.
- The PE drops to 1/2 (mid) or 1/3.7 (low) clock after any idle gap and
  needs 3us of continuous execution to re-reach full clock, so phase
  boundaries are engineered to keep the PE streak unbroken: S lives on the
  Pool engine, PSUM evictions are split DVE/Act, E groups run back-to-back.
"""

import numpy as np

import concourse.bass as bass
import concourse.tile as tile
from concourse import bacc, mybir
from concourse.bass_utils import run_bass_kernel_spmd

F32 = mybir.dt.float32
F16 = mybir.dt.float16
AX = mybir.AxisListType
AF = mybir.ActivationFunctionType
ALU = mybir.AluOpType

C = 512
N = 4096
P = 128
CT = C // P
NS = N // P
NSB = NS // 4       # stream superblocks: 4 x 128 rows per DMA
NCH = N // 512
NCORES = 8

# misc pack layout (f16, per-partition columns)
MK_KBB = 0          # [0:512)   kb broadcast
MK_ID = C           # [512:640) identity
MK_VB = C + P       # [640:644) vb blocks
MK_GAM = C + P + CT  # [644:646) gamma as raw f32 bytes
MISC_W = C + P + CT + 2

_cached = {}


def _build_program(repeat=1):
    from contextlib import ExitStack

    nc = bacc.Bacc("TRN2", target_bir_lowering=False, debug=False,
                   num_devices=NCORES)

    xT_d = nc.dram_tensor("xT", [N, C], F16, kind="ExternalInput").ap()
    qT_d = nc.dram_tensor("qT", [N, C], F16, kind="ExternalInput").ap()
    x_d = nc.dram_tensor("x", [C, N], F16, kind="ExternalInput").ap()
    kwT_d = nc.dram_tensor("kwT", [C, C], F16, kind="ExternalInput").ap()
    vwN_d = nc.dram_tensor("vwN", [C, C], F16, kind="ExternalInput").ap()
    misc_d = nc.dram_tensor("misc", [P, MISC_W], F16,
                            kind="ExternalInput").ap()
    out_d = nc.dram_tensor("out", [C, N], F16, kind="ExternalOutput").ap()

    # batched stream views
    xT_v = xT_d.rearrange("(b k p) c -> b p k c", k=4, p=P)
    qT_v = qT_d.rearrange("(b k p) c -> b p k c", k=4, p=P)
    xT_h = xT_d.rearrange("(b k p) c -> b p k c", k=1, p=P)   # single chunks
    qT_h = qT_d.rearrange("(b k p) c -> b p k c", k=1, p=P)
    xT_h2 = xT_d.rearrange("(b k p) c -> b p k c", k=2, p=P)
    qT_h2 = qT_d.rearrange("(b k p) c -> b p k c", k=2, p=P)
    x_v = x_d.rearrange("(eb p) (ch c) -> ch p eb c", eb=CT, p=P, ch=NCH)
    kwT_v = kwT_d.rearrange("(k p) c -> p k c", k=CT, p=P)
    vwN_v = vwN_d.rearrange("(k p) c -> p k c", k=CT, p=P)
    out_v = out_d.rearrange("(j p) n -> p j n", j=CT, p=P)

    with tile.TileContext(nc) as tc:
        with (
            tc.tile_pool(name="big", bufs=1) as big,
            tc.tile_pool(name="qp", bufs=4) as qp,
            tc.tile_pool(name="xtp", bufs=4) as xtp,
            tc.tile_pool(name="hp", bufs=4) as hp,
            tc.tile_pool(name="stp", bufs=3) as stp,
            tc.tile_pool(name="xcp", bufs=4) as xcp,
            tc.tile_pool(name="small", bufs=1) as small,
            tc.tile_pool(name="ps", bufs=1, space="PSUM") as ps,
        ):
            for _rep in range(repeat):
                kwT_sb = big.tile([P, CT, C], F16, tag="kw", name="kw")
                vw_sb = big.tile([P, CT, C], F16, tag="vw", name="vw")
                G_sb = [big.tile([P, C], F16, tag=f"g{c}", name=f"g{c}")
                        for c in range(CT)]
                a_sb = [big.tile([P, C], F16, tag=f"a{c}", name=f"a{c}")
                        for c in range(CT)]
                WT_sb = [big.tile([P, C], F16, tag=f"wt{c}", name=f"wt{c}")
                         for c in range(CT)]
                s_acc = [big.tile([P, C], F16, tag=f"sa{i}", name=f"sa{i}")
                         for i in range(2)]
                s16 = big.tile([P, C], F16, tag="s16", name="s16")
                misc_sb = small.tile([P, MISC_W], F16, tag="misc")
                kbb_sb = misc_sb[:, MK_KBB:MK_KBB + C]
                ident_sb = misc_sb[:, MK_ID:MK_ID + P]
                vb_sb = misc_sb[:, MK_VB:MK_VB + CT]
                gam_sb = misc_sb[:, MK_GAM:MK_GAM + 2].bitcast(F32)

                # ramp: nsb=0 split 1+1+2 chunks so the PE starts ~3us earlier
                xt0 = [hp.tile([P, 1, C], F16, tag="h1", name="xt0")
                       for _ in range(2)]
                qt0 = [hp.tile([P, 1, C], F16, tag="h1", name="qt0")
                       for _ in range(2)]
                xt0p = hp.tile([P, 2, C], F16, tag="h2", name="xt0p")
                qt0p = hp.tile([P, 2, C], F16, tag="h2", name="qt0p")
                nc.sync.dma_start(xt0[0], xT_h[0])
                nc.sync.dma_start(qt0[0], qT_h[0])
                nc.sync.dma_start(xt0[1], xT_h[1])
                nc.sync.dma_start(qt0[1], qT_h[1])
                nc.sync.dma_start(xt0p, xT_h2[1])
                nc.sync.dma_start(qt0p, qT_h2[1])
                nc.sync.dma_start(misc_sb, misc_d[:])
                xt1 = xtp.tile([P, 4, C], F16, tag="xt", name="xt")
                nc.sync.dma_start(xt1, xT_v[1])
                qt1 = qp.tile([P, 4, C], F16, tag="qt", name="qt")
                nc.sync.dma_start(qt1, qT_v[1])

                g_ps = [ps.tile([P, 512], F32, tag=f"pg{i}", name=f"gp{i}")
                        for i in range(CT)]
                xch = {}

                # ---- phase 1: G accumulation + S on DVE + x/weight streams
                for nsb in range(NSB):
                    if nsb == 6:
                        nc.sync.dma_start(kwT_sb, kwT_v)
                    if nsb == 7:
                        xch[0] = xcp.tile([P, CT, 512], F16, tag="xc",
                                          name="xc")
                        nc.sync.dma_start(xch[0], x_v[0])
                    if nsb == 0:
                        pass
                    elif nsb == 1:
                        xt, qt = xt1, qt1
                    else:
                        xt = xtp.tile([P, 4, C], F16, tag="xt", name="xt")
                        nc.sync.dma_start(xt, xT_v[nsb])
                        qt = qp.tile([P, 4, C], F16, tag="qt", name="qt")
                        nc.sync.dma_start(qt, qT_v[nsb])
                    for k in range(4):
                        ns = nsb * 4 + k
                        if nsb == 0:
                            if k < 2:
                                xs, qs = xt0[k][:, 0, :], qt0[k][:, 0, :]
                            else:
                                xs, qs = xt0p[:, k - 2, :], qt0p[:, k - 2, :]
                        else:
                            xs, qs = xt[:, k, :], qt[:, k, :]
                        for ct in range(CT):
                            nc.tensor.matmul(g_ps[ct][:],
                                             xs[:, ct * P:(ct + 1) * P],
                                             qs, start=(ns == 0),
                                             stop=(ns == NS - 1))
                        # S accumulation on the DVE (idle in phase 1);
                        # f16 all-through for the 2-elem/lane/cycle mode
                        if ns < 2:
                            nc.vector.tensor_copy(s_acc[ns][:], qs)
                        else:
                            nc.vector.tensor_tensor(
                                out=s_acc[ns % 2][:], in0=s_acc[ns % 2][:],
                                in1=qs, op=ALU.add)

                # s16 = acc0 + acc1 (f16; contracted against kbb by the PE)
                nc.vector.tensor_tensor(out=s16[:], in0=s_acc[0][:],
                                        in1=s_acc[1][:], op=ALU.add)

                # vw + x chunks 1,2 stream during phase 2 (DMA idle window)
                nc.sync.dma_start(vw_sb, vwN_v)
                for c in (1, 2):
                    xch[c] = xcp.tile([P, CT, 512], F16, tag="xc", name="xc")
                    nc.sync.dma_start(xch[c], x_v[c])

                # ---- evict G to SBUF (f16), split DVE/Act ----
                nc.vector.tensor_copy(G_sb[0][:], g_ps[0][:])
                nc.scalar.activation(G_sb[1][:], g_ps[1][:], AF.Copy)
                nc.vector.tensor_copy(G_sb[2][:], g_ps[2][:])
                nc.scalar.activation(G_sb[3][:], g_ps[3][:], AF.Copy)

                wt_ps = [ps.tile([P, 512], F32, tag=f"pw{i}", name=f"w{i}")
                         for i in range(CT)]
                e_ps = [ps.tile([P, 512], F32, tag=f"pg{i}", name=f"e{i}")
                        for i in range(CT)]

                # ---- E groups back-to-back on the PE ----
                for i in range(CT):
                    for ct in range(CT):
                        nc.tensor.matmul(e_ps[i][:],
                                         kwT_sb[:, ct, i * P:(i + 1) * P],
                                         G_sb[ct][:], start=(ct == 0),
                                         stop=False)
                    nc.tensor.matmul(e_ps[i][:], kbb_sb[:, i * P:(i + 1) * P],
                                     s16[:], start=False, stop=True)

                # softmax chases each e_ps stop; W matmuls chase each a_sb
                for i in range(CT):
                    nmx = small.tile([P, 1], F32, tag=f"nmx{i}", name=f"nmx{i}")
                    nc.vector.reduce_max(nmx[:], e_ps[i][:], axis=AX.X,
                                         negate=True)
                    ssum = small.tile([P, 1], F32, tag=f"ssum{i}",
                                      name=f"ssum{i}")
                    nc.scalar.activation(a_sb[i][:], e_ps[i][:], AF.Exp,
                                         bias=nmx[:, 0:1], scale=1.0,
                                         accum_out=ssum[:, 0:1])
                    rs = small.tile([P, 1], F32, tag=f"rs{i}", name=f"rs{i}")
                    nc.vector.reciprocal(rs[:], ssum[:])
                    nc.vector.tensor_scalar(
                        out=a_sb[i][:], in0=a_sb[i][:], scalar1=rs[:, 0:1],
                        scalar2=gam_sb[:, 0:1], op0=ALU.mult, op1=ALU.mult)
                    for eb in range(CT):
                        nc.tensor.matmul(wt_ps[eb][:],
                                         vw_sb[:, i, eb * P:(eb + 1) * P],
                                         a_sb[i][:], start=(i == 0),
                                         stop=(i == CT - 1))

                b2_full = ps.tile([P, 512], F32, tag="pg0", name="b2_ps")
                b2_ps = b2_full[:, 0:CT]

                # b2[j] = sum_d A[d,j] vb[d]: 16 one-column matmuls that keep
                # the PE busy while DVE/Act evict WT below
                for j in range(CT):
                    for i in range(CT):
                        nc.tensor.matmul(b2_ps[:, j:j + 1],
                                         a_sb[i][:, j * P:(j + 1) * P],
                                         vb_sb[:, i:i + 1], start=(i == 0),
                                         stop=(i == CT - 1))

                # ---- evict WT split DVE/Act; diagonal +I adds on DVE,
                # emitted right after each copy so WT_sb[0] (the first
                # stationary of the final phase) is ready earliest ----
                nc.scalar.activation(WT_sb[1][:], wt_ps[1][:], AF.Copy)
                nc.scalar.activation(WT_sb[3][:], wt_ps[3][:], AF.Copy)
                nc.vector.tensor_copy(WT_sb[0][:], wt_ps[0][:])
                nc.vector.tensor_tensor(
                    out=WT_sb[0][:, 0:P], in0=wt_ps[0][:, 0:P],
                    in1=ident_sb[:], op=ALU.add)
                nc.vector.tensor_tensor(
                    out=WT_sb[1][:, P:2 * P], in0=wt_ps[1][:, P:2 * P],
                    in1=ident_sb[:], op=ALU.add)
                nc.vector.tensor_copy(WT_sb[2][:], wt_ps[2][:])
                nc.vector.tensor_tensor(
                    out=WT_sb[2][:, 2 * P:3 * P], in0=wt_ps[2][:, 2 * P:3 * P],
                    in1=ident_sb[:], op=ALU.add)
                nc.vector.tensor_tensor(
                    out=WT_sb[3][:, 3 * P:4 * P], in0=wt_ps[3][:, 3 * P:4 * P],
                    in1=ident_sb[:], op=ALU.add)
                b2_sb = small.tile([P, CT], F32, tag="b2s")
                nc.vector.tensor_copy(b2_sb[:], b2_ps[:])

                # ---- final: out = (W+I)@x + b2 ----
                for ch in range(NCH):
                    if ch + 3 < NCH and ch + 3 >= 3:
                        c = ch + 3
                        xch[c] = xcp.tile([P, CT, 512], F16, tag="xc",
                                          name="xc")
                        nc.sync.dma_start(xch[c], x_v[c])
                    ot = stp.tile([P, CT, 512], F16, tag="ot", name="ot")
                    for j in range(CT):
                        o_ps = ps.tile([P, 512], F32,
                                       tag=f"pg{1 + (ch * CT + j) % 3}",
                                       name="o_ps")
                        for eb in range(CT):
                            nc.tensor.matmul(
                                o_ps[:], WT_sb[eb][:, j * P:(j + 1) * P],
                                xch[ch][:, eb, :],
                                start=(eb == 0), stop=(eb == CT - 1))
                        if ch == NCH - 1 and j % 2 == 1:
                            nc.vector.tensor_scalar_add(
                                ot[:, j, :], o_ps[:], b2_sb[:, j:j + 1])
                        else:
                            nc.scalar.activation(ot[:, j, :], o_ps[:],
                                                 AF.Identity,
                                                 bias=b2_sb[:, j:j + 1],
                                                 scale=1.0)
                        if ch == NCH - 1:
                            nc.sync.dma_start(
                                out_v[:, j:j + 1, ch * 512:(ch + 1) * 512],
                                ot[:, j:j + 1, :])
                    if ch < NCH - 1:
                        nc.sync.dma_start(out_v[:, :, ch * 512:(ch + 1) * 512],
                                          ot[:])

    nc.compile()
    return nc


def _get_program(repeat=1):
    if repeat not in _cached:
        _cached[repeat] = _build_program(repeat)
    return _cached[repeat]


def make_in_maps(x, proj_query, key_w, key_b, value_w, value_b, gamma):
    """Per-core input dicts: batch-parallel shards + replicated weights."""
    B = x.shape[0]
    xb = np.asarray(x, dtype=np.float32).reshape(B, C, N)
    x16 = [np.ascontiguousarray(xb[b], dtype=np.float16) for b in range(B)]
    xT16 = [np.ascontiguousarray(xb[b].T, dtype=np.float16) for b in range(B)]
    qT = np.ascontiguousarray(
        np.asarray(proj_query, np.float32).reshape(C, N).T, dtype=np.float16)
    kwT = np.ascontiguousarray(np.asarray(key_w).T, dtype=np.float16)
    vwN = np.ascontiguousarray(np.asarray(value_w), dtype=np.float16)
    misc = np.zeros((P, MISC_W), dtype=np.float16)
    misc[:, MK_KBB:MK_KBB + C] = np.asarray(key_b, np.float32).reshape(1, C)
    misc[:, MK_ID:MK_ID + P] = np.eye(P, dtype=np.float16)
    misc[:, MK_VB:MK_VB + CT] = np.asarray(
        value_b, np.float32).reshape(CT, P).T.astype(np.float16)
    misc[:, MK_GAM:MK_GAM + 2] = np.asarray(
        gamma, np.float32).reshape(1, 1).view(np.float16)
    return [
        {"x": x16[b], "xT": xT16[b], "qT": qT, "kwT": kwT, "vwN": vwN,
         "misc": misc}
        for b in range(B)
    ]


def kernel(x, proj_query, key_w, key_b, value_w, value_b, gamma, **_unused):
    B, Cx, W, H = x.shape
    assert (B, Cx, W * H) == (NCORES, C, N)
    nc = _get_program()
    in_maps = make_in_maps(x, proj_query, key_w, key_b, value_w, value_b,
                           gamma)
    res = run_bass_kernel_spmd(nc, in_maps, list(range(NCORES)))
    out = np.stack([res.results[b]["out"] for b in range(B)])
    return out.reshape(B, C, W, H).astype(np.float32)
